# revision 59
# baseline (speedup 1.0000x reference)
"""Distributed Trainium2 kernel for nn_Attention (RMSNorm + QKV + RoPE +
causal SDPA + out-proj) over 8 NeuronCores.

v5 strategy (Megatron head-sharding, AG-first):
  phase A: each core RMS-norms its own 512-token chunk (features on
           partitions), casts to bf16 and AllGathers the normalized
           activations (1MB/rank; the rendezvous happens early, and the
           weight DMAs/casts fill the wait).
  phase B: each core projects ALL 4096 tokens for its own 2 heads
           (wq/wk slices are host-permuted for de-interleaved RoPE; V is
           projected token-major so no PE transposes are needed),
           applies RoPE locally, and SDPA steps are emitted interleaved
           with the projections so exp (ACT) overlaps proj matmuls (PE).
           SDPA: scores for the two heads are computed back-to-back into
           one [128,1024] 2-bank PSUM tile (row-packed K=64 matmuls run
           concurrently on the two array halves), ONE exp per block pair
           covers both heads, diag-block masking via a dual-window 3D AP,
           AV matmul with a ones column in V so the softmax denominator
           accumulates in the same PSUM tile; deferred division.
  A2A:     context head-sharded -> token-sharded (1MB/rank).
  phase C: out-projection for the core's own 512-token chunk (full wo,
           staged during phase B).
Host does layout-only prep (transpose, per-core head-column slices,
constant RoPE/mask tables) and the final concat.
"""
import sys

sys.path.insert(0, "/opt/trn_rl_repo")

import numpy as np
import ml_dtypes
from contextlib import ExitStack

import concourse.bass as bass
import concourse.mybir as mybir
import concourse.tile as tile
from concourse import bacc
from concourse.bass_utils import run_bass_kernel_spmd

F32 = mybir.dt.float32
BF16 = mybir.dt.bfloat16

B, S, D, H, DH = 2, 2048, 1024, 16, 64
NC = 8
TOK = B * S            # 4096
CHUNK = TOK // NC      # 512
EPS = 1.1920929e-07
THETA = 10000.0
NKB = S // 128         # key blocks per batch: 16
QT = S // 512          # q tiles per batch: 4

_CACHE = {}
DEBUG_DUMP = False


def _build():
    nc = bacc.Bacc("TRN2", target_bir_lowering=False, debug=False, num_devices=NC)

    xt_d = nc.dram_tensor("xt", [D, CHUNK], F32, kind="ExternalInput")
    nw_d = nc.dram_tensor("nw", [D, 1], F32, kind="ExternalInput")
    wq_d = nc.dram_tensor("wqc", [D, 128], F32, kind="ExternalInput")
    wk_d = nc.dram_tensor("wkc", [D, 128], F32, kind="ExternalInput")
    wv_d = nc.dram_tensor("wvc", [D, 128], F32, kind="ExternalInput")
    wo_d = nc.dram_tensor("wo", [D, D], F32, kind="ExternalInput")
    cos_d = nc.dram_tensor("cosb", [128, S], BF16, kind="ExternalInput")
    sin_d = nc.dram_tensor("sinb", [128, S], BF16, kind="ExternalInput")
    msk_d = nc.dram_tensor("dmask", [128, 128], BF16, kind="ExternalInput")
    out_d = nc.dram_tensor("out", [CHUNK, D], F32, kind="ExternalOutput")

    if DEBUG_DUMP:
        dbg_q = nc.dram_tensor("dbg_q", [128, TOK], BF16, kind="ExternalOutput")
        dbg_k = nc.dram_tensor("dbg_k", [128, TOK], BF16, kind="ExternalOutput")
        dbg_v = nc.dram_tensor("dbg_v", [128, B * NKB * 130], BF16, kind="ExternalOutput")
        dbg_c = nc.dram_tensor("dbg_c", [128, TOK], BF16, kind="ExternalOutput")

    pm_d = nc.dram_tensor("permm", [128, 128], BF16, kind="ExternalInput")
    ag_in = nc.dram_tensor("ag_in", [D, CHUNK], BF16)
    ag_out = nc.dram_tensor("ag_out", [NC * D, CHUNK], BF16, addr_space="Shared")
    a2a_in = nc.dram_tensor("a2a_in", [NC * 128, CHUNK], BF16)
    a2a_out = nc.dram_tensor("a2a_out", [NC * 128, CHUNK], BF16)

    with tile.TileContext(nc) as tc, ExitStack() as ctx:
        pp = ctx.enter_context(tc.tile_pool(name="persist", bufs=1))

        # ---- persistent tiles ----
        qT = pp.tile([128, TOK], BF16, tag="qT")
        kT = pp.tile([128, TOK], BF16, tag="kT")
        v_all = pp.tile([128, B * NKB * 130], BF16, tag="v_all")
        cosT = pp.tile([128, S], BF16, tag="cosT")
        sinT = pp.tile([128, S], BF16, tag="sinT")
        dm2 = pp.tile([128, 256], BF16, tag="dm2")
        ones1 = pp.tile([1, 128], BF16, tag="ones1")
        ones128 = pp.tile([128, 1], BF16, tag="ones128")
        nw_sb = pp.tile([128, 8], F32, tag="nw_sb")
        permT = pp.tile([128, 128], BF16, tag="permT")
        wq_sb = pp.tile([128, 8, 128], BF16, tag="wq_sb")
        wk_sb = pp.tile([128, 8, 128], BF16, tag="wk_sb")
        wv_sb = pp.tile([128, 8, 128], BF16, tag="wv_sb")
        wo_sb = pp.tile([128, 8, 1024], BF16, tag="wo_sb")
        ctx_sb = pp.tile([128, TOK], BF16, tag="ctx_sb")
        vv = v_all.rearrange("p (blk c) -> p blk c", c=130)

        nc.vector.memset(ones1, 1.0)
        nc.vector.memset(ones128, 1.0)
        # ones columns of the extended V blocks (softmax denominator);
        # the V copies never overwrite cols 64/129.
        nc.gpsimd.memset(vv[:, :, 64:65], 1.0)
        nc.gpsimd.memset(vv[:, :, 129:130], 1.0)
        nc.sync.dma_start(out=nw_sb.rearrange("p (kt o) -> p kt o", o=1),
                          in_=nw_d.rearrange("(kt p) o -> p kt o", p=128))

        # ---- phase A: local RMSNorm of own chunk (transposed layout) ----
        sc0 = nc.named_scope("p0_rmsnorm"); sc0.__enter__()
        xn_cm = tc.tile_pool(name="xnc", bufs=1)
        xn_pool = xn_cm.__enter__()
        xn_sb = []
        with tc.tile_pool(name="rms", bufs=1) as rms_pool, \
             tc.tile_pool(name="ps0", bufs=1, space="PSUM") as ps0:
            ssq = ps0.tile([1, CHUNK], F32, tag="ssq")
            xbs = []
            for kt in range(8):
                xtile = rms_pool.tile([128, CHUNK], F32, tag=f"xt{kt}")
                nc.sync.dma_start(out=xtile, in_=xt_d[kt * 128 : (kt + 1) * 128, :])
                xbs.append(xtile)
                xsq = rms_pool.tile([128, CHUNK], BF16, tag="xsq")
                nc.vector.tensor_mul(xsq, xtile, xtile)
                nc.tensor.matmul(ssq, ones128, xsq, start=(kt == 0), stop=(kt == 7))
            eps_t = rms_pool.tile([1, 1], F32, tag="eps_t")
            nc.vector.memset(eps_t, float(EPS))
            rstd = rms_pool.tile([1, CHUNK], F32, tag="rstd")
            nc.scalar.activation(rstd, ssq, mybir.ActivationFunctionType.Sqrt,
                                 bias=eps_t[0:1, 0:1], scale=1.0 / D)
            inv = rms_pool.tile([1, CHUNK], F32, tag="inv")
            nc.vector.reciprocal_approx_fast(out=inv, in_=rstd)
            invb = rms_pool.tile([1, CHUNK], BF16, tag="invb")
            nc.vector.tensor_copy(invb, inv)
            rb = ps0.tile([128, CHUNK], F32, tag="rb")
            nc.tensor.matmul(rb, ones1, invb, start=True, stop=True)
            # xn = (x * nw) * 1/rms  (normalized local chunk, bf16)
            for kt in range(8):
                xn = xn_pool.tile([128, CHUNK], BF16, name=f"xn{kt}", tag=f"xn{kt}")
                nc.vector.scalar_tensor_tensor(
                    out=xn, in0=xbs[kt], scalar=nw_sb[:, kt : kt + 1], in1=rb,
                    op0=mybir.AluOpType.mult, op1=mybir.AluOpType.mult)
                xn_sb.append(xn)
                nc.sync.dma_start(out=ag_in[kt * 128 : (kt + 1) * 128, :], in_=xn)
        sc0.__exit__(None, None, None)

        # ---- weight staging + cast (DVE), tables; fills the AG wait ----
        sc1 = nc.named_scope("p1_stage"); sc1.__enter__()
        with tc.tile_pool(name="wstage", bufs=2) as wstage:
            for w_sb, d in ((wq_sb, wq_d), (wk_sb, wk_d), (wv_sb, wv_d)):
                st = wstage.tile([128, 8 * 128], F32, tag="wst")
                nc.sync.dma_start(out=st.rearrange("p (kt c) -> p kt c", c=128),
                                  in_=d.rearrange("(kt p) c -> p kt c", p=128))
                for kt in range(8):
                    nc.vector.tensor_copy(w_sb[:, kt, :], st[:, kt * 128 : (kt + 1) * 128])
        nc.sync.dma_start(out=cosT, in_=cos_d[:, :])
        nc.sync.dma_start(out=sinT, in_=sin_d[:, :])
        nc.sync.dma_start(out=dm2[:, 0:128], in_=msk_d[:, :])
        nc.sync.dma_start(out=dm2[:, 128:256], in_=msk_d[:, :])
        nc.sync.dma_start(out=permT, in_=pm_d[:, :])
        sc1.__exit__(None, None, None)

        # ---- AllGather: normalized activations to every core ----
        sc2 = nc.named_scope("p2_ag"); sc2.__enter__()
        nc.gpsimd.collective_compute(
            "AllGather", mybir.AluOpType.bypass,
            replica_groups=[list(range(NC))],
            ins=[ag_in.ap().opt()], outs=[ag_out.ap().opt()])
        sc2.__exit__(None, None, None)

        # ---- phase B: projections (2 own heads, all tokens) + RoPE,
        #      with SDPA steps emitted as their chunks become ready ----
        sc3 = nc.named_scope("p3_projsdpa"); sc3.__enter__()
        dm3 = dm2.rearrange("p (g c) -> p g c", c=128)

        # one SPMD program for all cores: uniform chunk order
        chunk_order = list(range(NC))
        # SDPA step (b, j) needs token chunks {b*4 .. b*4+j} projected
        steps = [(b, j) for b in range(B) for j in range(QT)]

        xg_cm = tc.tile_pool(name="xg", bufs=2)
        xg_pool = xg_cm.__enter__()
        pb_cm = tc.tile_pool(name="pb", bufs=4)
        pb_pool = pb_cm.__enter__()
        pexp_cm = tc.tile_pool(name="pexp", bufs=10)
        pexp = pexp_cm.__enter__()
        cn_cm = tc.tile_pool(name="cn", bufs=4)
        cn = cn_cm.__enter__()
        ps_sc_cm = tc.tile_pool(name="pssc", bufs=2, space="PSUM")
        ps_sc = ps_sc_cm.__enter__()
        ps_ctx_cm = tc.tile_pool(name="psctx", bufs=1, space="PSUM")
        ps_ctx = ps_ctx_cm.__enter__()
        wo_cm = tc.tile_pool(name="wostage", bufs=2)
        wostage = wo_cm.__enter__()
        ps_proj_cm = tc.tile_pool(name="psproj", bufs=1, space="PSUM")
        ps_proj = ps_proj_cm.__enter__()
        ps_vb_cm = tc.tile_pool(name="psvb", bufs=1, space="PSUM")
        ps_vb = ps_vb_cm.__enter__()

        done = set()
        pending_norm = []

        def rope_proj(w_sb, dst, xg, i):
            """project chunk i (features->128 head dims), apply RoPE into dst.
            The rotate-half partition swap is a PE permutation matmul (denser
            PE stream, no SBUF-SBUF DMAs)."""
            acc = ps_proj.tile([128, CHUNK], F32, name="aqk", tag="aqk")
            for kt in range(8):
                nc.tensor.matmul(acc, w_sb[:, kt, :], xg[:, kt * CHUNK : (kt + 1) * CHUNK],
                                 start=(kt == 0), stop=(kt == 7))
            t = pb_pool.tile([128, CHUNK], BF16, tag="rt")
            nc.vector.tensor_copy(t, acc)
            swp = ps_proj.tile([128, CHUNK], F32, name="swp", tag="aqk")
            nc.tensor.matmul(swp, permT, t, start=True, stop=True)
            pos0 = (i % 4) * CHUNK
            sw = pb_pool.tile([128, CHUNK], BF16, tag="rsw")
            nc.vector.tensor_mul(sw, swp, sinT[:, pos0 : pos0 + CHUNK])
            nc.vector.tensor_mul(t, t, cosT[:, pos0 : pos0 + CHUNK])
            nc.vector.tensor_add(dst[:, i * CHUNK : (i + 1) * CHUNK], t, sw)

        def proj_chunk(i):
            xg = xg_pool.tile([128, 8 * CHUNK], BF16, tag="xg")
            for kt in range(8):
                r0 = i * D + kt * 128
                nc.sync.dma_start(out=xg[:, kt * CHUNK : (kt + 1) * CHUNK],
                                  in_=ag_out[r0 : r0 + 128, :])
            rope_proj(wq_sb, qT, xg, i)
            rope_proj(wk_sb, kT, xg, i)
            # V token-major: [128 tokens, 128 dims] per 128-token block
            for tb in range(4):
                vp = ps_vb.tile([128, CHUNK], F32, name="vb", tag="vb")
                for kt in range(8):
                    nc.tensor.matmul(
                        vp[:, 0:128],
                        xg[:, kt * CHUNK + tb * 128 : kt * CHUNK + (tb + 1) * 128],
                        wv_sb[:, kt, :], start=(kt == 0), stop=(kt == 7))
                blk = i * 4 + tb
                nc.vector.tensor_copy(vv[:, blk, 0:64], vp[:, 0:64])
                nc.vector.tensor_copy(vv[:, blk, 65:129], vp[:, 64:128])

        def emit_norm(b, j, ctxp, bc_alloc=None):
            base = b * S
            for hi, r0 in ((0, 0), (1, 64)):
                den_s = cn.tile([1, CHUNK], F32, tag="den_s")
                nc.vector.tensor_copy(den_s, ctxp[hi][64:65, :])
                rec = cn.tile([1, CHUNK], F32, tag="rec")
                nc.vector.reciprocal_approx_fast(out=rec, in_=den_s)
                recb = cn.tile([1, CHUNK], BF16, tag="recb")
                nc.vector.tensor_copy(recb, rec)
                if bc_alloc is None:
                    bcp = ps_vb.tile([128, CHUNK], F32, name="bc", tag="vb")
                else:
                    bcp = bc_alloc()
                nc.tensor.matmul(bcp[0:64, 0:CHUNK], ones1[0:1, 0:64], recb, start=True, stop=True)
                bcs = cn.tile([64, CHUNK], BF16, tag="bcs")
                nc.vector.tensor_copy(bcs, bcp[0:64, 0:CHUNK])
                nc.vector.tensor_mul(
                    ctx_sb[r0 : r0 + 64, base + CHUNK * j : base + CHUNK * (j + 1)],
                    ctxp[hi][0:64, :], bcs)
            cch = b * QT + j
            nc.sync.dma_start(
                out=a2a_in[cch * 128 : cch * 128 + 64, :],
                in_=ctx_sb[0:64, cch * CHUNK : (cch + 1) * CHUNK])
            nc.sync.dma_start(
                out=a2a_in[cch * 128 + 64 : (cch + 1) * 128, :],
                in_=ctx_sb[64:128, cch * CHUNK : (cch + 1) * CHUNK])

        def emit_block(b, j, kb, ctxp, sc_pools=None):
            base = b * S
            nkb = 4 * (j + 1)
            m = kb - 4 * j
            c0 = 128 * m if m >= 0 else 0
            w = CHUNK - c0
            qcol0 = base + CHUNK * j + c0
            koff = base + kb * 128
            pool = ps_sc if sc_pools is None else sc_pools[kb % len(sc_pools)]
            sc = pool.tile([128, 1024], F32, name="sc", tag="sc")
            for hi, r0 in ((0, 0), (1, 64)):
                nc.tensor.matmul(
                    sc[:, hi * 512 + c0 : hi * 512 + 512],
                    kT[r0 : r0 + 64, koff : koff + 128],
                    qT[r0 : r0 + 64, qcol0 : qcol0 + w],
                    start=True, stop=True)
            p = pexp.tile([128, 1024], BF16, name="p", tag="p")
            nc.scalar.activation(p[:, c0:1024], sc[:, c0:1024],
                                 mybir.ActivationFunctionType.Exp, scale=0.125)
            if m >= 0:
                pw = p.rearrange("q (g c) -> q g c", c=512)[:, :, c0 : c0 + 128]
                nc.vector.tensor_mul(pw, pw, dm3)
            vcol = (b * NKB + kb) * 130
            for hi in range(2):
                nc.tensor.matmul(
                    ctxp[hi][:, c0:CHUNK],
                    v_all[:, vcol + hi * 65 : vcol + hi * 65 + 65],
                    p[:, hi * 512 + c0 : hi * 512 + 512],
                    start=(kb == 0), stop=(kb == nkb - 1),
                    skip_group_check=True)

        def emit_step(b, j, pool, tags):
            ctxp = {0: pool.tile([65, CHUNK], F32, name=tags[0], tag=tags[0]),
                    1: pool.tile([65, CHUNK], F32, name=tags[1], tag=tags[1])}
            for kb in range(4 * (j + 1)):
                emit_block(b, j, kb, ctxp)
            return ctxp

        def stage_wo(kt):
            st2 = wostage.tile([128, 1024], F32, tag="wost")
            nc.sync.dma_start(out=st2[0:64, :], in_=wo_d[kt * 128 : kt * 128 + 64, :])
            nc.sync.dma_start(out=st2[64:128, :], in_=wo_d[kt * 128 + 64 : (kt + 1) * 128, :])
            nc.vector.tensor_copy(wo_sb[:, kt, :], st2)

        # steps run as their chunk prerequisites complete, interleaved with
        # the projections; the final step (1,3) runs post-proj with a
        # 3-deep score-buffer rotation (PSUM freed by the proj pools)
        singles = {0: (0, 0), 1: (0, 1), 2: (0, 2), 3: (0, 3),
                   4: (1, 0), 5: (1, 1), 6: (1, 2)}
        for i in chunk_order:
            proj_chunk(i)
            done.add(i)
            # flush deferred normalizes (after next chunk's DVE ops)
            while pending_norm:
                nb, nj, nctx = pending_norm.pop(0)
                emit_norm(nb, nj, nctx)
            stage_wo(i)  # one wo tile per iteration: spreads DMA + DVE load
            if i in singles:
                b, j = singles[i]
                ctxp = emit_step(b, j, ps_ctx, ("ctxA", "ctxB"))
                pending_norm.append((b, j, ctxp))

        while pending_norm:
            nb, nj, nctx = pending_norm.pop(0)
            emit_norm(nb, nj, nctx)

        # projections done: release their PSUM, run the final step
        ps_vb_cm.__exit__(None, None, None)
        ps_proj_cm.__exit__(None, None, None)
        ps_sc2_cm = tc.tile_pool(name="pssc2", bufs=1, space="PSUM")
        ps_sc2 = ps_sc2_cm.__enter__()
        ctx1 = {0: ps_ctx.tile([65, CHUNK], F32, name="ctxA", tag="ctxA"),
                1: ps_ctx.tile([65, CHUNK], F32, name="ctxB", tag="ctxB")}
        for kb in range(16):
            emit_block(1, 3, kb, ctx1, sc_pools=[ps_sc, ps_sc, ps_sc2])
        emit_norm(1, 3, ctx1,
                  bc_alloc=lambda: ps_sc2.tile([128, 1024], F32, name="bc", tag="sc"))

        ps_sc2_cm.__exit__(None, None, None)
        wo_cm.__exit__(None, None, None)
        ps_ctx_cm.__exit__(None, None, None)
        ps_sc_cm.__exit__(None, None, None)
        cn_cm.__exit__(None, None, None)
        pexp_cm.__exit__(None, None, None)
        pb_cm.__exit__(None, None, None)
        xg_cm.__exit__(None, None, None)
        xn_cm.__exit__(None, None, None)
        sc3.__exit__(None, None, None)

        # ---- phase C pools + PE warm-keeper across the A2A window ----
        sc6 = nc.named_scope("p6_outproj"); sc6.__enter__()
        with tc.tile_pool(name="ctxgp", bufs=1) as ctxgp, \
             tc.tile_pool(name="outp", bufs=3) as outp, \
             tc.tile_pool(name="ps6", bufs=2, space="PSUM") as ps6:
            # ---- A2A: head-sharded ctx -> token-sharded ctx ----
            sc5 = nc.named_scope("p5_a2a2"); sc5.__enter__()
            nc.gpsimd.collective_compute(
                "AllToAll", mybir.AluOpType.bypass,
                replica_groups=[list(range(NC))],
                ins=[a2a_in.ap().opt()], outs=[a2a_out.ap().opt()])
            sc5.__exit__(None, None, None)

            # ---- phase C: out-projection on own token chunk ----
            ctxg = ctxgp.tile([128, NC * CHUNK], BF16, tag="ctxg")
            for cb in range(NC):
                nc.sync.dma_start(out=ctxg[0:64, cb * CHUNK : (cb + 1) * CHUNK],
                                  in_=a2a_out[cb * 128 : cb * 128 + 64, :])
                nc.sync.dma_start(out=ctxg[64:128, cb * CHUNK : (cb + 1) * CHUNK],
                                  in_=a2a_out[cb * 128 + 64 : (cb + 1) * 128, :])
            for tl in range(4):
                pso = {nh: ps6.tile([128, 512], F32, name=f"op{nh}", tag=f"op{nh}") for nh in range(2)}
                for nh in range(2):
                    for cb in range(8):
                        nc.tensor.matmul(
                            pso[nh],
                            ctxg[:, cb * CHUNK + tl * 128 : cb * CHUNK + (tl + 1) * 128],
                            wo_sb[:, cb, nh * 512 : (nh + 1) * 512],
                            start=(cb == 0), stop=(cb == 7))
                ost = outp.tile([128, 1024], F32, tag="ost")
                nc.scalar.copy(ost[:, 0:512], pso[0])
                nc.scalar.copy(ost[:, 512:1024], pso[1])
                nc.sync.dma_start(out=out_d[tl * 128 : tl * 128 + 64, :], in_=ost[0:64, :])
                nc.sync.dma_start(out=out_d[tl * 128 + 64 : (tl + 1) * 128, :], in_=ost[64:128, :])
        sc6.__exit__(None, None, None)

        if DEBUG_DUMP:
            nc.sync.dma_start(out=dbg_q[:, :], in_=qT)
            nc.sync.dma_start(out=dbg_k[:, :], in_=kT)
            nc.sync.dma_start(out=dbg_v[:, :], in_=v_all)
            nc.sync.dma_start(out=dbg_c[:, :], in_=ctx_sb)

    nc.compile()
    return nc


def _head_cols(h, deinterleave):
    base = h * DH
    if deinterleave:
        return np.concatenate([base + np.arange(0, DH, 2), base + np.arange(1, DH, 2)])
    return base + np.arange(DH)


def _make_tables():
    inv_freq = 1.0 / (THETA ** (np.arange(0, DH, 2) / DH))   # [32]
    ang = np.arange(S)[:, None] * inv_freq[None, :]          # [2048, 32]
    ch = np.cos(ang).T.astype(np.float32)                    # [32, 2048]
    sh = np.sin(ang).T.astype(np.float32)
    cosb = np.concatenate([ch, ch, ch, ch], axis=0)          # [128, 2048]
    sinb = np.concatenate([-sh, sh, -sh, sh], axis=0)
    kk, qq = np.meshgrid(np.arange(128), np.arange(128), indexing="ij")
    dmask = (kk <= qq).astype(np.float32)
    bf = ml_dtypes.bfloat16
    return cosb.astype(bf), sinb.astype(bf), dmask.astype(bf)


def _in_maps(inputs):
    x = np.ascontiguousarray(inputs["x"], dtype=np.float32)
    norm_w = np.asarray(inputs["norm_w"], dtype=np.float32)
    wq = np.asarray(inputs["wq"], dtype=np.float32)
    wk = np.asarray(inputs["wk"], dtype=np.float32)
    wv = np.asarray(inputs["wv"], dtype=np.float32)
    wo = np.ascontiguousarray(inputs["wo"], dtype=np.float32)

    xT = np.ascontiguousarray(x.reshape(TOK, D).T)           # [1024, 4096]
    cosb, sinb, dmask = _make_tables()
    nw = np.ascontiguousarray(norm_w.reshape(D, 1))
    # rotate-half partition swap as a (symmetric) permutation matrix
    swap = np.concatenate([np.arange(32, 64), np.arange(0, 32),
                           np.arange(96, 128), np.arange(64, 96)])
    perm = np.zeros((128, 128), np.float32)
    perm[np.arange(128), swap] = 1.0
    perm = perm.astype(ml_dtypes.bfloat16)

    maps = []
    for c in range(NC):
        qcols = np.concatenate([_head_cols(2 * c, True), _head_cols(2 * c + 1, True)])
        vcols = np.concatenate([_head_cols(2 * c, False), _head_cols(2 * c + 1, False)])
        maps.append({
            "xt": np.ascontiguousarray(xT[:, c * CHUNK : (c + 1) * CHUNK]),
            "nw": nw,
            "wqc": np.ascontiguousarray(wq[:, qcols]),
            "wkc": np.ascontiguousarray(wk[:, qcols]),
            "wvc": np.ascontiguousarray(wv[:, vcols]),
            "wo": wo,
            "cosb": cosb,
            "sinb": sinb,
            "dmask": dmask,
            "permm": perm,
        })
    return maps


def _run(inputs, trace=False):
    if "ncs" not in _CACHE:
        _CACHE["ncs"] = _build()
    nc = _CACHE["ncs"]
    res = run_bass_kernel_spmd(nc, _in_maps(inputs), core_ids=list(range(NC)),
                               trace=trace)
    chunks = [res.results[c]["out"] for c in range(NC)]
    out = np.concatenate(chunks, axis=0).reshape(B, S, D).astype(np.float32)
    return out, res


def kernel(**inputs) -> np.ndarray:
    out, _ = _run(inputs, trace=False)
    return out
